# revision 16
# baseline (speedup 1.0000x reference)
"""MipNeRF resampling (inverse-CDF sampling + conical frustum -> Gaussian)
as a TRN2 Bass/Tile kernel, data-parallel over rays across 8 NeuronCores.

Algorithm (gather-free inverse-CDF):
  The sample grid u_s = s*step is FIXED (linspace), so each CDF fencepost's
  insertion position among the u's is closed-form. Work in a scaled domain
  (cdfS = cdf/step, so u_s -> s): start_j = ceil(cdfS_j). Segment j covers
  output samples [start_j, start_{j+1}); each non-empty segment's
  interpolation coefficients (binsL, cdfSL, 1/dcdfS, dbins) are scattered to
  free-dim position start_j with a per-partition GPSIMD local_scatter (fp32
  packed as int16 halves), forward-filled with DVE tensor_tensor_scan, and
  all samples evaluated elementwise:
      t = clamp((s - C)*Q, 0, 1);  sample = B + t*D
  Work is split across DVE / ACT (per-block scale+bias APs for the
  outer-product means/covs) / Pool to run all engines in parallel.
"""
import numpy as np
from contextlib import ExitStack

import concourse.bass as bass
import concourse.tile as tile
import concourse.bacc as bacc
import concourse.mybir as mybir
from concourse.bass_utils import run_bass_kernel_spmd

f32 = mybir.dt.float32
i16 = mybir.dt.int16
i32 = mybir.dt.int32
Alu = mybir.AluOpType
Act = mybir.ActivationFunctionType
AX = mybir.AxisListType

B = 16384            # total rays
NCORES = 8
BC = B // NCORES     # rays per core (2048)
P = 128              # partitions
NBLKS = BC // P      # 16 ray-blocks of 128 rays per core
NB = 4               # blocks per round
ROUNDS = NBLKS // NB
S = 129              # fenceposts / samples per ray
NSEG = 128           # segments per ray
NQ = 4               # scattered quantities (B, C, Q, D)
DSTW = S * NQ        # fp32 scatter-dst width per block (516)

# u_s = s * STEP exactly reproduces jnp.linspace(0, 1-eps, 129) (verified)
UEPS = float(np.finfo(np.float32).eps)
STEP = np.float32((1.0 - UEPS) / 128.0)
INV_STEP = np.float32(np.float32(1.0) / STEP)
MAGIC = float(np.float32(2.0 ** 23))

_prog_cache = {}


def _build_program():
    nc = bacc.Bacc("TRN2", target_bir_lowering=False, debug=False,
                   enable_asserts=False, num_devices=NCORES)

    tv_d = nc.dram_tensor("tv_in", [BC, S], f32, kind="ExternalInput").ap()
    wt_d = nc.dram_tensor("wt_in", [BC, NSEG], f32, kind="ExternalInput").ap()
    org_d = nc.dram_tensor("org_in", [BC, 3], f32, kind="ExternalInput").ap()
    dir_d = nc.dram_tensor("dir_in", [BC, 3], f32, kind="ExternalInput").ap()
    rad_d = nc.dram_tensor("rad_in", [BC, 1], f32, kind="ExternalInput").ap()

    newt_d = nc.dram_tensor("newt_out", [BC, S], f32, kind="ExternalOutput").ap()
    mean_d = nc.dram_tensor("mean_out", [BC, NSEG * 3], f32, kind="ExternalOutput").ap()
    cov_d = nc.dram_tensor("cov_out", [BC, NSEG * 3], f32, kind="ExternalOutput").ap()

    # ray r = block*128 + p  ->  [p, block, feat]
    tv_v = tv_d.rearrange("(b p) f -> p b f", p=P)
    wt_v = wt_d.rearrange("(b p) f -> p b f", p=P)
    org_v = org_d.rearrange("(b p) f -> p b f", p=P)
    dir_v = dir_d.rearrange("(b p) f -> p b f", p=P)
    rad_v = rad_d.rearrange("(b p) f -> p b f", p=P)
    newt_v = newt_d.rearrange("(b p) f -> p b f", p=P)
    mean_v = mean_d.rearrange("(b p) f -> p b f", p=P)
    cov_v = cov_d.rearrange("(b p) f -> p b f", p=P)

    with tile.TileContext(nc) as tc, ExitStack() as ctx:
        cpool = ctx.enter_context(tc.tile_pool(name="const", bufs=1))
        inp = ctx.enter_context(tc.tile_pool(name="inp", bufs=2))
        wk = ctx.enter_context(tc.tile_pool(name="wk", bufs=1))
        wk2 = ctx.enter_context(tc.tile_pool(name="wk2", bufs=2))
        outp = ctx.enter_context(tc.tile_pool(name="outp", bufs=2))

        # ---------------- constants ----------------
        iota_i = cpool.tile([P, NB * S], i32, tag="iota_i")
        nc.gpsimd.iota(iota_i[:], pattern=[[0, NB], [1, S]], base=0,
                       channel_multiplier=0)
        iota_f = cpool.tile([P, NB * S], f32, tag="iota_f")
        nc.vector.tensor_copy(iota_f[:], iota_i[:])
        # resetm: 0.0 at the first slot of each block, 1.0 elsewhere
        resetm = cpool.tile([P, NB * S], f32, tag="resetm")
        nc.vector.tensor_scalar(resetm[:], iota_f[:], 0.5, None, Alu.is_ge)
        # iota07 int16 (0..7) for index expansion
        iota07_i = cpool.tile([P, 8], i32, tag="iota07_i")
        nc.gpsimd.iota(iota07_i[:], pattern=[[1, 8]], base=0, channel_multiplier=0)
        iota07 = cpool.tile([P, 8], i16, tag="iota07")
        nc.vector.tensor_copy(iota07[:], iota07_i[:])

        bias001 = cpool.tile([P, 1], f32, tag="bias001")
        nc.vector.memset(bias001[:], 0.01)

        # pdfx slot-0: tiny negative so cdfS_0 = -1e-7 breaks the rne(1.5)
        # tie in startp1 = rne(cdfS + 1.5) toward 1 = ceil(0)+1. The whole
        # cumsum shifts by -1e-7, which only moves exact-tie assignments
        # (benign by continuity) and biases t by <= 1e-7*Q ~ 1e-5.
        pdfx = cpool.tile([P, NB, S], f32, tag="pdfx")
        nc.vector.memset(pdfx[:, :, 0:1], -1e-7)

        for r in range(ROUNDS):
            bsl = slice(r * NB, (r + 1) * NB)

            # ---------------- loads ----------------
            tv = inp.tile([P, NB, S], f32, tag="tv")
            nc.sync.dma_start(tv[:], tv_v[:, bsl])
            wt = inp.tile([P, NB, NSEG + 1], f32, tag="wt")
            nc.sync.dma_start(wt[:, :, 0:NSEG], wt_v[:, bsl])
            org = inp.tile([P, NB, 3], f32, tag="org")
            nc.sync.dma_start(org[:], org_v[:, bsl])
            dirs = inp.tile([P, NB, 3], f32, tag="dirs")
            nc.sync.dma_start(dirs[:], dir_v[:, bsl])
            rad = inp.tile([P, NB, 1], f32, tag="rad")
            nc.sync.dma_start(rad[:], rad_v[:, bsl])

            # ---------------- weight blur ----------------
            # wmax[t] = max(wpad[t], wpad[t+1]); edge-padded
            wmax = wk2.tile([P, NB, S], f32, tag="wmax")
            nc.vector.tensor_copy(wt[:, :, NSEG:NSEG + 1], wt[:, :, NSEG - 1:NSEG])
            nc.vector.tensor_copy(wmax[:, :, 0:1], wt[:, :, 0:1])
            nc.vector.tensor_tensor(wmax[:, :, 1:S], wt[:, :, 0:NSEG],
                                    wt[:, :, 1:NSEG + 1], Alu.max)
            wq = wk2.tile([P, NB, NSEG], f32, tag="wq")
            nc.gpsimd.tensor_tensor(wq[:], wmax[:, :, 0:NSEG], wmax[:, :, 1:S],
                                    Alu.add)
            # wq = 0.5*(sum) + 0.01
            nc.scalar.activation(wq[:], wq[:], Act.Identity, scale=0.5,
                                 bias=bias001[:])

            # ---------------- pdf (scaled by 1/step) ----------------
            wsum = wk.tile([P, NB], f32, tag="wsum")
            nc.vector.tensor_reduce(wsum[:], wq[:], AX.X, Alu.add)
            padw = wk.tile([P, NB], f32, tag="padw")
            nc.vector.tensor_scalar(padw[:], wsum[:], -1.0, 1e-5, Alu.mult, Alu.add)
            nc.vector.tensor_scalar(padw[:], padw[:], 0.0, None, Alu.max)
            nc.vector.scalar_tensor_tensor(
                wq[:], padw[:].unsqueeze(2).broadcast_to([P, NB, NSEG]),
                1.0 / NSEG, wq[:], Alu.mult, Alu.add)
            nc.vector.tensor_tensor(wsum[:], wsum[:], padw[:], Alu.add)
            invw = wk.tile([P, NB], f32, tag="invw")
            nc.vector.reciprocal(invw[:], wsum[:])
            invwS = wk.tile([P, NB], f32, tag="invwS")
            nc.vector.tensor_scalar(invwS[:], invw[:], float(INV_STEP), None,
                                    Alu.mult)
            # pdfx[:, b, 1:S] = wq * invwS[b]  (per-block scale on ACT)
            for b in range(NB):
                nc.scalar.activation(pdfx[:, b, 1:S], wq[:, b, :], Act.Identity,
                                     scale=invwS[:, b:b + 1])

            # ---------- scaled cdf (scan, per-block reset) + start ----------
            cdfS = wk2.tile([P, NB, S], f32, tag="cdfS")
            nc.vector.tensor_tensor_scan(
                cdfS[:].rearrange("p a b -> p (a b)"), resetm[:],
                pdfx[:].rearrange("p a b -> p (a b)"), 0.0, Alu.mult, Alu.add)
            nc.vector.memset(cdfS[:, :, S - 1:S], float(INV_STEP))
            # startp1 = ceil(cdfS)+1 = rne(cdfS + 1.5) (ties benign)
            sh = wk2.tile([P, NB, S], f32, tag="sh")
            nc.vector.tensor_scalar(sh[:], cdfS[:], 1.5, None, Alu.add)
            startf = wk2.tile([P, NB, S], f32, tag="startf")
            nc.vector.tensor_scalar(startf[:], sh[:], MAGIC, MAGIC, Alu.add,
                                    Alu.subtract)

            # ---------------- segment coefficients, packed ----------------
            dpack = wk2.tile([P, NB * NSEG, NQ], f32, tag="dpack")
            dp3 = dpack[:].rearrange("p (a j) q -> p a j q", j=NSEG)
            nc.scalar.copy(dp3[:, :, :, 0], tv[:, :, 0:NSEG])          # B = binsL
            nc.scalar.copy(dp3[:, :, :, 1], cdfS[:, :, 0:NSEG])        # C = cdfSL
            dcdf = wk2.tile([P, NB, NSEG], f32, tag="dcdf")
            nc.gpsimd.tensor_tensor(dcdf[:], cdfS[:, :, 1:S], cdfS[:, :, 0:NSEG],
                                    Alu.subtract)
            nc.vector.reciprocal_approx_fast(
                dp3[:, :, :, 2].rearrange("p a j -> p (a j)").unsqueeze(2)
                .rearrange("p t o -> p (t o)"),
                dcdf[:].rearrange("p a j -> p (a j)"))                 # Q = 1/dcdfS
            nc.gpsimd.tensor_tensor(dp3[:, :, :, 3], tv[:, :, 1:S],
                                    tv[:, :, 0:NSEG], Alu.subtract)    # D = dbins

            # ---------------- scatter indices ----------------
            valid = wk2.tile([P, NB, NSEG], f32, tag="valid")
            nc.vector.tensor_tensor(valid[:], startf[:, :, 0:NSEG],
                                    startf[:, :, 1:S], Alu.is_lt)
            sf8 = wk2.tile([P, NB, NSEG], f32, tag="sf8")
            nc.vector.tensor_tensor(sf8[:], startf[:, :, 0:NSEG], valid[:],
                                    Alu.mult)
            nc.vector.tensor_scalar(sf8[:], sf8[:], 8.0, -8.0, Alu.mult, Alu.add)
            sf8i = wk2.tile([P, NB * NSEG], i16, tag="sf8i")
            nc.vector.tensor_copy(sf8i[:], sf8[:].rearrange("p a b -> p (a b)"))
            idxi = wk2.tile([P, NB * NSEG, 8], i16, tag="idxi")
            nc.vector.tensor_tensor(
                idxi[:],
                sf8i[:].unsqueeze(2).broadcast_to([P, NB * NSEG, 8]),
                iota07[:].unsqueeze(1).broadcast_to([P, NB * NSEG, 8]),
                Alu.add)

            # ---------------- scatter + forward fill ----------------
            dst = wk2.tile([P, NB * DSTW], f32, tag="dst")
            for bn in range(NB):
                nc.gpsimd.local_scatter(
                    dst[:, bn * DSTW:(bn + 1) * DSTW].bitcast(i16),
                    dpack[:, bn * NSEG:(bn + 1) * NSEG, :]
                        .rearrange("p a q -> p (a q)").bitcast(i16),
                    idxi[:, bn * NSEG:(bn + 1) * NSEG, :]
                        .rearrange("p a q -> p (a q)"),
                    channels=P, num_elems=2 * DSTW, num_idxs=2 * NQ * NSEG)

            dstq = dst[:].rearrange("p (t q) -> p t q", q=NQ)
            oneM = wk2.tile([P, NB * S], f32, tag="oneM")
            nc.gpsimd.tensor_scalar(oneM[:], dstq[:, :, 0], 0.0, None, Alu.is_le)
            FB = wk2.tile([P, NB * S], f32, tag="FB")
            nc.vector.tensor_tensor_scan(FB[:], oneM[:], dstq[:, :, 0], 0.0,
                                         Alu.mult, Alu.add)
            FC = wk2.tile([P, NB * S], f32, tag="FC")
            nc.vector.tensor_tensor_scan(FC[:], oneM[:], dstq[:, :, 1], 0.0,
                                         Alu.mult, Alu.add)
            FQ = wk2.tile([P, NB * S], f32, tag="FQ")
            nc.vector.tensor_tensor_scan(FQ[:], oneM[:], dstq[:, :, 2], 0.0,
                                         Alu.mult, Alu.add)
            FD = wk2.tile([P, NB * S], f32, tag="FD")
            nc.vector.tensor_tensor_scan(FD[:], oneM[:], dstq[:, :, 3], 0.0,
                                         Alu.mult, Alu.add)

            # ---------------- evaluate samples ----------------
            xs = wk2.tile([P, NB * S], f32, tag="xs")
            nc.gpsimd.tensor_tensor(xs[:], iota_f[:], FC[:], Alu.subtract)
            nc.vector.tensor_tensor(xs[:], xs[:], FQ[:], Alu.mult)
            samples = outp.tile([P, NB, S], f32, tag="samples")
            sflat = samples[:].rearrange("p a b -> p (a b)")
            nc.vector.tensor_tensor(sflat, xs[:], FD[:], Alu.mult)
            nc.gpsimd.tensor_tensor(sflat, sflat, FB[:], Alu.add)
            nc.sync.dma_start(newt_v[:, bsl], samples[:])

            # ---------------- conical frustum -> gaussian ----------------
            # scaled: s_=2mu, d_=2hw, den=4*denom, e=d2*R
            t0 = samples[:, :, 0:NSEG]
            t1 = samples[:, :, 1:S]
            s_ = wk.tile([P, NB, NSEG], f32, tag="s_")
            nc.gpsimd.tensor_tensor(s_[:], t0, t1, Alu.add)
            d_ = wk.tile([P, NB, NSEG], f32, tag="d_")
            nc.gpsimd.tensor_tensor(d_[:], t1, t0, Alu.subtract)
            s2 = wk.tile([P, NB, NSEG], f32, tag="s2")
            nc.scalar.activation(s2[:], s_[:], Act.Square)
            d2 = wk.tile([P, NB, NSEG], f32, tag="d2")
            nc.scalar.activation(d2[:], d_[:], Act.Square)
            den = wk.tile([P, NB, NSEG], f32, tag="den")
            nc.vector.scalar_tensor_tensor(den[:], s2[:], 3.0, d2[:],
                                           Alu.mult, Alu.add)
            R = wk.tile([P, NB, NSEG], f32, tag="R")
            nc.vector.reciprocal_approx_fast(
                R[:].rearrange("p a b -> p (a b)"),
                den[:].rearrange("p a b -> p (a b)"))
            e_ = wk.tile([P, NB, NSEG], f32, tag="e_")
            nc.gpsimd.tensor_tensor(e_[:], d2[:], R[:], Alu.mult)
            A2 = wk.tile([P, NB, NSEG], f32, tag="A2")
            nc.vector.tensor_tensor(A2[:], s_[:], e_[:], Alu.mult)
            t_mean = wk2.tile([P, NB, NSEG], f32, tag="t_mean")
            nc.vector.scalar_tensor_tensor(t_mean[:], s_[:], 0.5, A2[:],
                                           Alu.mult, Alu.add)
            q_ = wk.tile([P, NB, NSEG], f32, tag="q_")
            nc.vector.tensor_scalar(q_[:], d2[:], -1.25, None, Alu.mult)
            nc.gpsimd.tensor_tensor(q_[:], q_[:], den[:], Alu.add)     # 3s2-.25d2
            w1 = wk.tile([P, NB, NSEG], f32, tag="w1")
            nc.scalar.activation(w1[:], e_[:], Act.Square)             # d4*R^2
            m_ = wk.tile([P, NB, NSEG], f32, tag="m_")
            nc.gpsimd.tensor_tensor(m_[:], w1[:], q_[:], Alu.mult)
            tvar = wk2.tile([P, NB, NSEG], f32, tag="tvar")
            nc.vector.scalar_tensor_tensor(tvar[:], m_[:], -16.0 / 5.0, d2[:],
                                           Alu.mult, Alu.add)          # 12*t_var
            n1 = wk.tile([P, NB, NSEG], f32, tag="n1")
            nc.gpsimd.tensor_tensor(n1[:], d2[:], e_[:], Alu.mult)     # d4*R
            u1 = wk.tile([P, NB, NSEG], f32, tag="u1")
            nc.vector.tensor_scalar(u1[:], d2[:], 5.0 / 3.0, None, Alu.mult)
            nc.gpsimd.tensor_tensor(u1[:], u1[:], s2[:], Alu.add)
            rvar = wk2.tile([P, NB, NSEG], f32, tag="rvar")
            nc.vector.tensor_scalar(rvar[:], n1[:], -16.0 / 15.0, None, Alu.mult)
            nc.gpsimd.tensor_tensor(rvar[:], rvar[:], u1[:], Alu.add)  # 16*inner
            radsc = wk.tile([P, NB], f32, tag="radsc")
            nc.vector.tensor_tensor(radsc[:], rad[:, :, 0], rad[:, :, 0], Alu.mult)
            nc.vector.tensor_scalar(radsc[:], radsc[:], 1.0 / 16.0, None, Alu.mult)
            for b in range(NB):                                        # r_var
                nc.scalar.activation(rvar[:, b, :], rvar[:, b, :], Act.Identity,
                                     scale=radsc[:, b:b + 1])

            # direction-dependent per-ray vectors
            d2v = wk.tile([P, NB, 3], f32, tag="d2v")
            nc.vector.tensor_tensor(d2v[:], dirs[:], dirs[:], Alu.mult)
            dmag = wk.tile([P, NB], f32, tag="dmag")
            nc.vector.tensor_reduce(dmag[:], d2v[:], AX.X, Alu.add)
            nc.vector.tensor_scalar(dmag[:], dmag[:], 1e-10, None, Alu.add)
            invm = wk.tile([P, NB], f32, tag="invm")
            nc.vector.reciprocal(invm[:], dmag[:])
            nullv = wk.tile([P, NB, 3], f32, tag="nullv")
            nc.vector.tensor_tensor(nullv[:], d2v[:],
                                    invm[:].unsqueeze(2).broadcast_to([P, NB, 3]),
                                    Alu.mult)
            nc.vector.tensor_scalar(nullv[:], nullv[:], -1.0, 1.0, Alu.mult,
                                    Alu.add)                           # 1 - d2/dmag
            d2vs = wk.tile([P, NB, 3], f32, tag="d2vs")
            nc.vector.tensor_scalar(d2vs[:], d2v[:], 1.0 / 12.0, None, Alu.mult)

            # -------- means / covs: per-(block, i) scale+bias on ACT --------
            means = outp.tile([P, NB, NSEG, 3], f32, tag="means")
            covs = outp.tile([P, NB, NSEG, 3], f32, tag="covs")
            for b in range(NB):
                for i in range(3):
                    nc.scalar.activation(
                        means[:, b, :, i], t_mean[:, b, :], Act.Identity,
                        scale=dirs[:, b, i:i + 1], bias=org[:, b, i:i + 1])
                    nc.scalar.activation(
                        covs[:, b, :, i], tvar[:, b, :], Act.Identity,
                        scale=d2vs[:, b, i:i + 1])
                    ct = wk.tile([P, NSEG], f32, tag="ct")
                    nc.gpsimd.tensor_scalar(
                        ct[:], rvar[:, b, :],
                        nullv[:, b, i:i + 1].rearrange("p o -> p o"),
                        None, Alu.mult)
                    nc.gpsimd.tensor_tensor(covs[:, b, :, i], covs[:, b, :, i],
                                            ct[:], Alu.add)
            nc.sync.dma_start(mean_v[:, bsl],
                              means[:].rearrange("p a j c -> p a (j c)"))
            nc.sync.dma_start(cov_v[:, bsl],
                              covs[:].rearrange("p a j c -> p a (j c)"))

    nc.compile()
    return nc


def _get_program():
    if "nc" not in _prog_cache:
        _prog_cache["nc"] = _build_program()
    return _prog_cache["nc"]


def kernel(origins, directions, radii, t_vals, weights, _trace=False, **_kw):
    nc = _get_program()
    origins = np.ascontiguousarray(origins, np.float32)
    directions = np.ascontiguousarray(directions, np.float32)
    radii = np.ascontiguousarray(radii, np.float32)
    t_vals = np.ascontiguousarray(t_vals, np.float32)
    weights = np.ascontiguousarray(weights, np.float32)

    in_maps = []
    for c in range(NCORES):
        sl = slice(c * BC, (c + 1) * BC)
        in_maps.append({
            "tv_in": t_vals[sl], "wt_in": weights[sl], "org_in": origins[sl],
            "dir_in": directions[sl], "rad_in": radii[sl],
        })
    res = run_bass_kernel_spmd(nc, in_maps, list(range(NCORES)), trace=_trace)
    newt = np.concatenate([res.results[c]["newt_out"] for c in range(NCORES)], 0)
    means = np.concatenate([res.results[c]["mean_out"] for c in range(NCORES)], 0)
    covs = np.concatenate([res.results[c]["cov_out"] for c in range(NCORES)], 0)
    out = (newt, means.reshape(B, NSEG, 3), covs.reshape(B, NSEG, 3))
    if _trace:
        return out, res
    return out


if __name__ == "__main__":
    rng = np.random.default_rng(0)
    d = rng.standard_normal((B, 3)).astype(np.float32)
    smoke = kernel(
        origins=rng.standard_normal((B, 3)).astype(np.float32),
        directions=(d / np.linalg.norm(d, axis=1, keepdims=True)),
        radii=rng.uniform(0.001, 0.005, (B, 1)).astype(np.float32),
        t_vals=np.sort(rng.uniform(2, 6, (B, S)).astype(np.float32), axis=1),
        weights=rng.uniform(0, 1, (B, NSEG)).astype(np.float32),
    )
    print([x.shape for x in smoke])
